# revision 3
# baseline (speedup 1.0000x reference)
"""LSTM encoder kernel for Trainium2 (8 NeuronCores, data-parallel over batch).

v2: PE column-tiling. The per-step recurrent matmul out[32, 4096] wastes 3/4 of
the PE array with a [128, 32] stationary. Instead, 4 concurrent matmuls at PE
col-groups (tile_position (0, 32j)) compute 4 gate-chunks at once into one PSUM
bank [128, 512]: partitions 32j..32j+32 hold chunk m = 4r + j for batch rows b.

Gate-column permutation: n' = 512*m + 128*gi + rr  <->  orig 4H row gi*H + u,
u = 128*m + rr (PyTorch gate order i,f,g,o).  Chunk m = [i|f|g|o] x H-group m.
Round r in {0,1} -> chunks 4r..4r+3 -> one PSUM bank; elementwise runs on
[128, 128] tiles at full lane utilization.  Input projection is injected into
PSUM by a one-hot matmul (stationary = one-hot of idx[b,t] [120, 32], moving =
XC chunk [120, 512]).

h is transposed for the next step's stationary by a PE transpose (f32, via
identity) into PSUM + a gpsimd cast-copy to bf16 SBUF -- the DMA-transpose
path costs ~1.2us of Sync-engine descriptor generation per [32,128] block and
serializes the whole step.  The two transposes are pipelined into the PE
stream: T(r1 of t-1) between step t's k0-3 and k4-7 waves of round 0, and
T(r0 of t) before round 1's k4-7 waves.

Outputs are staged in DRAM as [steps, 128, 2*128] (the on-chip layout) with one
contiguous DMA per tensor per step; the host unpermutes to [BL, steps, H].
  c_sb [128, 2, 128] f32 : c[b, u],  p = 32j+b, u = 128*(4r+j)+rr
  hTT  [128, 2, 128] bf16: hTT[uu, r, 32j+b] = h[b, 128*(4r+j)+uu]
"""

from contextlib import ExitStack

import ml_dtypes
import numpy as np

import concourse.bacc as bacc
import concourse.bass as bass
import concourse.mybir as mybir
import concourse.tile as tile
from concourse.bass_utils import run_bass_kernel_spmd

F32 = mybir.dt.float32
BF16 = mybir.dt.bfloat16

B, S, E, H = 256, 256, 256, 1024
NCORES = 8
BL = B // NCORES          # 32 batch rows per core
NK = H // 128             # 8 contraction tiles
NR = 2                    # rounds (PSUM banks) per step
NJ = 4                    # concurrent PE col-groups

_cache = {}


def _build(steps: int):
    nc = bacc.Bacc("TRN2", target_bir_lowering=False, debug=False, enable_asserts=True)

    w_dram = nc.dram_tensor("W", [H, 4 * H], BF16, kind="ExternalInput")
    xc_dram = nc.dram_tensor("XC", [120, 4 * H], BF16, kind="ExternalInput")
    oh_dram = nc.dram_tensor("OH", [120, steps * BL], BF16, kind="ExternalInput")
    id_dram = nc.dram_tensor("ID", [128, 128], F32, kind="ExternalInput")
    hid_dram = nc.dram_tensor("hid", [steps, 128, NR * 128], F32,
                              kind="ExternalOutput")
    cell_dram = nc.dram_tensor("cell", [steps, 128, NR * 128], F32,
                               kind="ExternalOutput")

    Tanh = mybir.ActivationFunctionType.Tanh
    Sigmoid = mybir.ActivationFunctionType.Sigmoid

    with tile.TileContext(nc) as tc, ExitStack() as ctx:
        resident = ctx.enter_context(tc.tile_pool(name="resident", bufs=1))
        psum_pool = ctx.enter_context(tc.tile_pool(name="psum", bufs=1, space="PSUM"))
        act_pool = ctx.enter_context(tc.tile_pool(name="act", bufs=4))
        h_pool = ctx.enter_context(tc.tile_pool(name="h", bufs=4))

        w_sb = resident.tile([128, NK, 4 * H], BF16)
        w_view = w_dram.ap().rearrange("(k p) n -> k p n", p=128)
        for k in range(NK):
            eng = nc.sync if k % 2 == 0 else nc.scalar
            eng.dma_start(w_sb[:, k], w_view[k])
        xc_sb = resident.tile([120, 4 * H], BF16)
        nc.sync.dma_start(xc_sb[:], xc_dram[:])
        oh_sb = resident.tile([120, steps * BL], BF16)
        nc.scalar.dma_start(oh_sb[:], oh_dram[:])
        id_sb = resident.tile([128, 128], F32)
        nc.sync.dma_start(id_sb[:], id_dram[:])

        # 4 gate banks (double-buffered pairs) + transpose slots: c^T and o^T
        # share one bank per slot (PSUM tiles are bank-granular)
        ps_q = [psum_pool.tile([128, 512], F32, name=f"psq{i}") for i in range(4)]
        ps_to = [psum_pool.tile([128, 256], F32, name=f"psto{i}") for i in range(4)]

        c_st = [resident.tile([128, NR, 128], F32, name=f"c{i}") for i in range(3)]
        hTT_st = [resident.tile([128, NR, 128], BF16, name=f"hTT{i}")
                  for i in range(2)]

        def wave(p, src_stat, k_or_none, r, start, stop):
            for j in range(NJ):
                m = NJ * r + j
                if k_or_none is None:
                    stat, mov = src_stat, xc_sb[:, 512 * m:512 * (m + 1)]
                else:
                    k = k_or_none
                    stat = src_stat[:, k // NJ, 32 * (k % NJ):32 * (k % NJ) + 32]
                    mov = w_sb[:, k, 512 * m:512 * (m + 1)]
                nc.tensor.matmul(p[32 * j:32 * (j + 1), :], stat, mov,
                                 start=start, stop=stop,
                                 tile_position=(0, 32 * j))

        _o_of = {}

        def elementwise(p, r, t, c_new, c_old):
            g_t = act_pool.tile([128, 128], F32, tag="g", name=f"g{t}_{r}")
            if_t = act_pool.tile([128, 256], F32, tag="if", name=f"if{t}_{r}")
            o_t = act_pool.tile([128, 128], F32, tag="o", name=f"o{t}_{r}")
            _o_of[(t, r)] = o_t
            nc.scalar.activation(g_t[:], p[:, 256:384], Tanh)
            nc.scalar.activation(if_t[:], p[:, 0:256], Sigmoid)
            nc.scalar.activation(o_t[:], p[:, 384:512], Sigmoid)  # off-chain
            if t == 0:
                nc.vector.tensor_mul(c_new[:, r], if_t[:, 0:128], g_t[:])
            else:
                t1 = act_pool.tile([128, 128], F32, tag="t1", name=f"t1{t}_{r}")
                nc.vector.tensor_mul(t1[:], if_t[:, 0:128], g_t[:])
                nc.vector.tensor_mul(c_new[:, r], if_t[:, 128:256], c_old[:, r])
                nc.vector.tensor_add(c_new[:, r], c_new[:, r], t1[:])

        def transpose_h(t, r):
            # hTT_st[(t+1)%2][:, r] = sigmoid(o)^T * tanh(c^T): transpose c and
            # o on the PE, tanh in transposed space, one DVE mul -> bf16 hTT
            slot = ps_to[2 * (t % 2) + r]
            pt = slot[:, 0:128]
            po = slot[:, 128:256]
            nc.tensor.transpose(pt[:], c_st[t % 3][:, r], id_sb[:])
            nc.tensor.transpose(po[:], _o_of[(t, r)][:], id_sb[:])
            thT = act_pool.tile([128, 128], F32, tag="thT", name=f"thT{t}_{r}")
            nc.scalar.activation(thT[:], pt[:], Tanh)
            nc.vector.tensor_mul(hTT_st[(t + 1) % 2][:, r], po[:], thT[:])

        # inject waves for t=0 (the loop emits step t+1's injects during step t
        # so they can fill the PE stall while step t's r0 chain completes)
        for r in range(NR):
            wave(ps_q[r], oh_sb[:, 0:BL], None, r, start=True, stop=True)

        for t in range(steps):
            c_old = c_st[(t + 2) % 3]   # written at t-1
            c_new = c_st[t % 3]
            hTT_cur = hTT_st[t % 2]

            p0 = ps_q[2 * (t % 2)]
            p1 = ps_q[2 * (t % 2) + 1]
            if t > 0:
                for k in range(NJ):                       # k = 0..3
                    wave(p0, hTT_cur, k, 0, start=False, stop=False)
                transpose_h(t - 1, 1)                     # fills hTT_cur[:, 1]
                for k in range(NJ, NK):                   # k = 4..7
                    wave(p0, hTT_cur, k, 0, start=False, stop=(k == NK - 1))
                for k in range(NJ):
                    wave(p1, hTT_cur, k, 1, start=False, stop=False)
                for k in range(NJ, NK):
                    wave(p1, hTT_cur, k, 1, start=False, stop=(k == NK - 1))

            elementwise(p0, 0, t, c_new, c_old)
            elementwise(p1, 1, t, c_new, c_old)

            if t > 0:
                # h(t-1) fully transposed now (r1 part at top of this step)
                nc.gpsimd.dma_start(hid_dram[t - 1],
                                    hTT_st[t % 2][:].rearrange("p r x -> p (r x)"))

            # next step's inject waves: no h dependency, keep PE busy while
            # this step's r0 elementwise chain finishes
            if t + 1 < steps:
                oh_n = oh_sb[:, BL * (t + 1):BL * (t + 2)]
                for r in range(NR):
                    wave(ps_q[2 * ((t + 1) % 2) + r], oh_n, None, r,
                         start=True, stop=False)
            transpose_h(t, 0)                             # fills hTT_nxt[:, 0]

            nc.sync.dma_start(cell_dram[t], c_new[:].rearrange("p r x -> p (r x)"))

            _o_of.pop((t - 2, 0), None)
            _o_of.pop((t - 2, 1), None)

        transpose_h(steps - 1, 1)
        nc.gpsimd.dma_start(hid_dram[steps - 1],
                            hTT_st[steps % 2][:].rearrange("p r x -> p (r x)"))

    nc.compile()
    return nc


def _host_prep(letter_seq, state_seq, letter_emb, state_emb, W_ih, W_hh, b_ih, b_hh,
               steps: int):
    letter_seq = np.asarray(letter_seq)
    state_seq = np.asarray(state_seq)
    letter_emb = np.asarray(letter_emb, dtype=np.float32)
    state_emb = np.asarray(state_emb, dtype=np.float32)
    W_ih = np.asarray(W_ih, dtype=np.float32)
    W_hh = np.asarray(W_hh, dtype=np.float32)
    b_ih = np.asarray(b_ih, dtype=np.float32)
    b_hh = np.asarray(b_hh, dtype=np.float32)

    # col n' = 512*m + 128*gi + rr -> orig 4H row gi*H + 128*m + rr
    n = np.arange(4 * H)
    m, gi, rr = n // 512, (n % 512) // 128, n % 128
    colmap = gi * H + 128 * m + rr

    Wp = np.ascontiguousarray(W_hh[colmap, :].T).astype(ml_dtypes.bfloat16)  # [H, 4H]

    XL = letter_emb @ W_ih[:, :E].T                            # [30, 4H]
    XS = state_emb @ W_ih[:, E:].T                             # [4, 4H]
    bias = b_ih + b_hh
    XC = (XL[:, None, :] + XS[None, :, :] + bias).reshape(120, 4 * H)
    XC = np.ascontiguousarray(XC[:, colmap]).astype(ml_dtypes.bfloat16)

    eye = np.eye(128, dtype=np.float32)

    idx = (letter_seq.astype(np.int64) * 4 + state_seq.astype(np.int64))  # [B, S]
    in_maps = []
    for c in range(NCORES):
        idx_c = idx[BL * c:BL * (c + 1), :steps]               # [BL, steps]
        oh = np.zeros((120, steps * BL), dtype=ml_dtypes.bfloat16)
        cols = np.arange(steps * BL)
        oh[idx_c.T.reshape(-1), cols] = 1.0                    # col = t*BL + b
        in_maps.append({"W": Wp, "XC": XC, "OH": oh, "ID": eye})
    return in_maps


def _unstack(arr, steps):
    # [steps, 128, 256] (p = 32j+b, col = 128r+rr; u = 512r+128j+rr)
    # -> [BL, steps, H]
    a = arr.reshape(steps, NJ, BL, NR, 128)
    return np.ascontiguousarray(a.transpose(2, 0, 3, 1, 4).reshape(BL, steps, H))


def _unstack_t(arr, steps):
    # transposed layout: [steps, 128, 256] = [t, uu, (r, 32j+b)] -> [BL, steps, H]
    a = arr.reshape(steps, 128, NR, NJ, BL)
    return np.ascontiguousarray(a.transpose(4, 0, 2, 3, 1).reshape(BL, steps, H))


def run(letter_seq, state_seq, letter_emb, state_emb, W_ih, W_hh, b_ih, b_hh,
        steps: int = S, trace: bool = False):
    if steps not in _cache:
        _cache[steps] = _build(steps)
    nc = _cache[steps]

    in_maps = _host_prep(letter_seq, state_seq, letter_emb, state_emb,
                         W_ih, W_hh, b_ih, b_hh, steps)
    res = run_bass_kernel_spmd(nc, in_maps, core_ids=list(range(NCORES)),
                               trace=trace)

    hidden = np.concatenate([_unstack_t(res.results[c]["hid"], steps)
                             for c in range(NCORES)], axis=0)
    cell = np.concatenate([_unstack(res.results[c]["cell"], steps)
                           for c in range(NCORES)], axis=0)
    return (hidden, cell), res


def kernel(letter_seq, state_seq, letter_emb, state_emb, W_ih, W_hh, b_ih, b_hh,
           steps: int = S):
    (hidden, cell), _ = run(letter_seq, state_seq, letter_emb, state_emb,
                            W_ih, W_hh, b_ih, b_hh, steps)
    return hidden, cell
